# revision 24
# baseline (speedup 1.0000x reference)
"""Trainium2 Bass kernel for nn_AttentionConv (B=4,H=W=64,C=128,heads=2).

Sharding: 8 cores = (batch b in 0..3) x (query-half qh in 0..1).
Each core computes full attention for its 2048 query pixels of batch b,
over all 4096 keys, both heads, plus the qkv and output 1x1-conv
projections.  No cross-core communication.

v2 structure (vs the 174.9us ones-trick baseline): the PV matmuls for
the two heads are COLUMN-TILED (tile_position (0,0)/(0,64), M=64 each,
own XBUS stream per column group) so both heads' PV runs in ONE
512-cycle pass; the softmax row-sums Z come from a separate 4-way
column-tiled ones-matmul pass (M=32 per tile) every TWO key chunks,
streaming both heads of both chunks concurrently.  PE inner work drops
from 1536 to 1280 cycles/chunk.  has_written clear was verified
REGION-scoped on HW (probe.py), so each column tile manages its own
start= flag independently.

The exp split is unchanged: ScalarE ACT (exact exp, bias=ln G) and the
custom 8-stage DVE op EXP2_PACK_ANT (approx G*2^(T/128) packing bf16
bits through an int16-convert write) split the key chunks; scores
arrive pre-scaled by 128*log2(e)*C^-0.5 folded into wq host-side.

PSUM (8 banks): st pool 3 rotating [128,1024] f32 slots (6 banks);
o_ps [128,512] 1 bank (h0 dims on partitions 0-63, h1 on 64-127);
one shared "z" bank hosting, sequentially per qg: Z accumulation ->
zbc broadcast -> outproj gp -> next qg's Z accumulation.

Normalize for qg n (spread over qg n+1's first slots):
  kc0: orr = DVE copy o_ps (frees the o bank so PV lag stays 4);
       zsb = ACT copy z_ps (frees the z bank)
  kc2: zsum = zsb[0:64] + zsb[64:128] -> bf16 [64,512] (DVE)
  kc4: zbc = col-tiled K=1 ones-matmul pair broadcasting Z_h0 row ->
       partitions 0-63 and Z_h1 row -> 64-127 (PE, bf16)
  kc5: rz = reciprocal_approx_fast(zbc) (DVE)
  kc6: rt[:, qg] = orr * rz (DVE, SBUF x SBUF)
  kc9: outproj gp: K=1 ones x b_out bias init (start=True) + 4
       accumulating [128,128] matmuls (PE)
  kc10: ob = ACT copy(gp); kc12: out DMA.
"""

import numpy as np

import concourse.bass as bass
import concourse.tile as tile
from concourse.tile import add_dep_helper
from concourse import bacc, mybir
from concourse.bass_utils import run_bass_kernel_spmd

import concourse.dve_ops as dops
from concourse.dve_spec import Spec, lower, Src0, Src1, C0, C1, C2, Latch
from concourse.dve_uop import DveOpSpec

F32 = mybir.dt.float32
BF16 = mybir.dt.bfloat16
I16 = mybir.dt.int16

B = 4
C = 128
NPIX = 4096          # 64*64 pixels per batch
NQ = 2048            # queries per core (half batch)
HC = 64              # head dim
KC = 128             # key chunk
NKC = NPIX // KC     # 32
QG = 512             # query group (per head; ST tile packs both heads)
NQG = NQ // QG       # 4
N_CORES = 8
LAG_B = 2            # PV lags the S^T by LAG_B two-chunk batches
ET_BUFS = 16

# --- exp approximation constants (see fit in session notes) ---
EXP2_A = 2.63111957e-3
EXP2_B = 1.00227837
EXP2_C = 16310.5756
EXP2_M = float(1.5 * 2**30)
EXP2_BA = float(np.float32(EXP2_B) / np.float32(EXP2_A))
EXP2_LNG = 0.35459771189588246        # ln(G): ACT-path bias
LN2_128 = float(np.log(2.0) / 128.0)  # ACT-path scale
# host-side score pre-scale folded into wq: T = 128*log2(e)*C^-0.5 * s
PRESCALE = float(128.0 * np.log2(np.e) * C ** -0.5)

OP_NAME = "EXP2_PACK_ANT"

# which key-chunks each query-group runs on DVE (rest go to ACT)
import os as _os
_DVE_ON = _os.environ.get("KERNEL_DVE", "1") == "1"
_QG0_DVE = tuple(range(6, 32, 2))
_QGN_DVE = (1, 3, 5, 7, 9, 11, 15, 17, 19, 21, 23, 25, 27, 29, 31)
DVE_KCS = [
    _QG0_DVE if _DVE_ON else (),
    _QGN_DVE if _DVE_ON else (),
    _QGN_DVE if _DVE_ON else (),
    _QGN_DVE if _DVE_ON else (),
]

_CACHE = {}


def _exp2_ref(in0, in1, s0, s1, imm2):
    f32 = np.float32
    T = in0.astype(f32)
    u = f32(T + f32(s1))
    I = f32(u - f32(s1))
    F = f32(T - I)
    ba = np.asarray(in1, f32).reshape(in0.shape[0], -1)[:, :1]
    r = f32(F + ba)
    p = f32(F * r)
    Q = f32(p * f32(s0))
    return f32(f32(I + f32(imm2)) + Q)


def _register_exp2_op():
    for op in dops.OPS:
        if op.name == OP_NAME:
            return op
    u = Src0 + C1
    I = u - C1
    F = Src0 - I
    r = F + Latch(Src1)
    y = (I + C2) + (F * r) * C0
    spec = Spec(body=y, reference=_exp2_ref)
    row = dops._CUSTOM_DVE_ROW_BASE + len(dops.OPS)
    assert row < 0x20
    shas = {
        ver: DveOpSpec(
            name=OP_NAME, opcode=row, uops=lower(spec, ver=ver), rd1_en=True
        ).sha(ver)
        for ver in ("v3", "v4")
    }
    op = dops.DveOp(OP_NAME, spec, subdim=False, uops_sha=shas)
    dops.OPS.append(op)
    dops._SUB_OPCODE_FOR_NAME[OP_NAME] = row
    dops.CUSTOM_DVE_SPECS[OP_NAME] = spec
    return op


EXP2_OP = _register_exp2_op()


def _build_nc():
    nc = bacc.Bacc("TRN2", target_bir_lowering=False, debug=False)

    xt_d = nc.dram_tensor("xt", [C, NPIX], F32, kind="ExternalInput")
    w4_d = nc.dram_tensor("w4", [C, 512], F32, kind="ExternalInput")
    bo_d = nc.dram_tensor("bo", [1, C], F32, kind="ExternalInput")
    out_d = nc.dram_tensor("out", [NQ, C], F32, kind="ExternalOutput")

    Exp = mybir.ActivationFunctionType.Exp

    with tile.TileContext(nc) as tc:
        with (
            tc.tile_pool(name="const", bufs=1) as const,
            tc.tile_pool(name="stage", bufs=4) as stage,
            tc.tile_pool(name="et", bufs=ET_BUFS) as etp,
            tc.tile_pool(name="nrm", bufs=2) as nrm,
            tc.tile_pool(name="osb", bufs=2) as osbp,
            tc.tile_pool(name="st", bufs=1, space="PSUM") as stp,
            tc.tile_pool(name="op", bufs=1, space="PSUM") as opp,
            tc.tile_pool(name="zp", bufs=1, space="PSUM") as zpp,
        ):
            # ---- persistent SBUF tensors
            xtball = const.tile([C, NPIX], BF16, tag="xtball")
            qt = [const.tile([128, 512], BF16, tag=f"qt{j}", name=f"qt{j}")
                  for j in range(4)]
            kt = [const.tile([128, 512], BF16, tag=f"kt{j}", name=f"kt{j}")
                  for j in range(8)]
            vall = const.tile([128, NKC, 128], BF16, tag="vall")
            rt = const.tile([128, NQ], BF16, tag="rt")
            w4b = const.tile([C, 512], BF16, tag="w4b")
            wqb = w4b[:, 0:128]
            wkb = w4b[:, 128:256]
            wvb = w4b[:, 256:384]
            wob = w4b[:, 384:512]
            ones_zb = const.tile([128, 64], BF16, tag="ones_zb")
            ones1b = const.tile([1, 128], BF16, tag="ones1b")
            bo4b = const.tile([1, 512], BF16, tag="bo4b")
            warm = const.tile([1, 2], F32, tag="warm")
            ba_t = const.tile([128, 1], F32, tag="ba_t")
            lng_t = const.tile([128, 1], F32, tag="lng_t")

            # dummy exp first: loads the ACT table set off the critical path
            nc.vector.memset(warm[:], 0.0)
            nc.scalar.activation(warm[:], warm[:], Exp)
            nc.vector.memset(ba_t[:], EXP2_BA)
            nc.vector.memset(lng_t[:], EXP2_LNG)

            # PE warm-up: dummy matmuls while DMAs run, so the HAM
            # clock-gate reaches K=8/8 before the real matmuls start
            junk = const.tile([C, 512], BF16, tag="junk")
            nc.vector.memset(junk[:], 0.25)
            wst = stp.tile([128, 2 * QG], F32, tag="st", bufs=3, name="warm_st")
            for w in range(6):
                nc.tensor.matmul(wst[:, 0:512], junk[:, 0:128], junk[:],
                                 start=True, stop=True)

            # inputs: f32 HWDGE DMAs (x chunk 0 and weights first), casts
            # spread across DVE / GpSimd so no engine serializes the ramp
            xsall = stage.tile([C, NPIX], F32, tag="xsall", name="xsall")
            xs = [xsall[:, j * 512:(j + 1) * 512] for j in range(8)]
            # two HWDGE queues: SP carries x0 + mid-x, the Activation
            # engine's queue carries the weights + upper-x in parallel
            nc.sync.dma_start(xs[0], xt_d[:, 0:512])
            w32 = stage.tile([C, 512], F32, tag="w32", name="w32")
            nc.scalar.dma_start(w32[:], w4_d[:])
            nc.sync.dma_start(xsall[:, 512:2048], xt_d[:, 512:2048])
            nc.scalar.dma_start(xsall[:, 2048:4096], xt_d[:, 2048:4096])
            nc.vector.tensor_copy(w4b[:], w32[:])
            cast_eng = [nc.vector, nc.vector, nc.gpsimd, nc.gpsimd,
                        nc.gpsimd, nc.gpsimd, nc.gpsimd, nc.gpsimd]
            for j in range(8):
                eng = cast_eng[j]
                dst = xtball[:, j * 512:(j + 1) * 512]
                eng.tensor_copy(dst, xs[j])

            nc.vector.memset(ones_zb[:], 1.0)
            nc.gpsimd.memset(ones1b[:], 1.0)

            def emit_proj_kq(j):
                # QT/KT projections for one 512-pixel chunk; PSUM tiles
                # borrow ST-pool slots.
                p = stp.tile([128, 2 * QG], F32, tag="st", bufs=3,
                             name=f"pkq{j}")
                nc.tensor.matmul(p[:, 0:512], wkb,
                                 xtball[:, j * 512:(j + 1) * 512],
                                 start=True, stop=True)
                if j < 4:  # QT over local queries
                    nc.tensor.matmul(p[:, 512:1024], wqb,
                                     xtball[:, j * 512:(j + 1) * 512],
                                     start=True, stop=True)
                if j % 2 == 1:
                    nc.vector.tensor_copy(kt[j][:], p[:, 0:512])
                else:
                    nc.scalar.copy(kt[j][:], p[:, 0:512])
                if j < 4:
                    nc.vector.tensor_copy(qt[j][:], p[:, 512:1024])

            def emit_proj_v(j):
                pv = stp.tile([128, 2 * QG], F32, tag="st", bufs=3,
                              name=f"pv{j}")
                for kq in range(4):   # V natural per key chunk of 128
                    nc.tensor.matmul(
                        pv[:, kq * 128:(kq + 1) * 128],
                        xtball[:, j * 512 + kq * 128:
                               j * 512 + (kq + 1) * 128],
                        wvb, start=True, stop=True)
                dst = vall[:, 4 * j:4 * j + 4, :]
                src = pv[:, 0:512].rearrange("p (s d) -> p s d", d=128)
                if j % 2 == 1:
                    nc.vector.tensor_copy(dst, src)
                else:
                    nc.scalar.copy(dst, src)

            def emit_proj_chunk(j):
                emit_proj_kq(j)
                emit_proj_v(j)

            # ramp ordering: K/Q of chunk 0 first (feeds first S^T)
            emit_proj_kq(0)
            emit_proj_v(0)
            emit_proj_kq(1)
            emit_proj_v(1)
            bo32 = stage.tile([1, C], F32, tag="bo32")
            nc.sync.dma_start(bo32[:], bo_d[:])
            for r in range(4):
                nc.gpsimd.tensor_copy(bo4b[:, r * 128:(r + 1) * 128],
                                      bo32[:])

            # ---- attention (software-pipelined across query groups) ----
            def emit_zpair(qg, zsb):
                # DMA-realign the four Z rows (lane-locked engines cannot
                # cross partitions): rows {0,64} -> {0,1}, {32,96} -> {32,33}
                zpair = nrm.tile([34, QG], BF16, tag="zpr",
                                 name=f"zpair{qg}")
                nc.sync.dma_start(zpair[0:2, :], zsb[0:65:64, :])
                nc.sync.dma_start(zpair[32:34, :], zsb[32:97:64, :])
                return zpair

            def emit_zbc(qg, zpair):
                # bf16 K=2 ones-matmul: parity sum + broadcast in one pass;
                # Z_h0 -> partitions 0-63, Z_h1 -> 64-127 (concurrent pair
                # on disjoint rows/cols/psum partitions).
                zbc = zpp.tile([128, QG], F32, tag="z", name=f"zbc{qg}")
                nc.tensor.matmul(zbc[0:64, :], ones_zb[0:2, :],
                                 zpair[0:2, :], start=True, stop=True,
                                 tile_position=(0, 0), skip_group_check=True)
                nc.tensor.matmul(zbc[64:128, :], ones_zb[32:34, :],
                                 zpair[32:34, :], start=True, stop=True,
                                 tile_position=(32, 64),
                                 skip_group_check=True)
                return zbc

            def emit_outproj_mm(qg, anchor):
                q0 = qg * QG
                gp = zpp.tile([128, QG], F32, tag="z", name=f"gp{qg}")
                mm = nc.tensor.matmul(gp[:], ones1b[:], bo4b[:],
                                      start=True, stop=False,
                                      skip_group_check=True)
                if anchor is not None:
                    add_dep_helper(mm.ins, anchor.ins, False,
                                   "outproj after current S^T")
                for i in range(4):
                    nc.tensor.matmul(
                        gp[:, i * 128:(i + 1) * 128],
                        rt[:, q0 + i * 128:q0 + (i + 1) * 128],
                        wob, start=False, stop=(i == 3),
                        skip_group_check=True)
                return gp

            def emit_out_dma(qg, ob):
                q0 = qg * QG
                nc.sync.dma_start(
                    out_d[q0:q0 + QG, :].rearrange("(c r) w -> r c w", r=128),
                    ob[:].rearrange("p (c w) -> p c w", w=128))

            # Z-pass schedule: batch index -> list of chunk-pair indices.
            # Pairs 0-5 wait for the z bank to clear the previous qg's
            # zbc/gp tenants (batch 5), then one pair per batch (matching
            # the exp supply rate).
            NB = NKC // 2 + 2   # 16 S^T batches + 2 PV/Z tail batches
            zsched = {5: (0, 1), 6: (2, 3), 7: (4, 5)}
            zsched.update({b: (b - 2,) for b in range(8, NB)})

            # pending state from the previous query group
            pend = None
            for qg in range(NQG):
                o_ps = opp.tile([128, QG], F32, tag="o", name=f"o_ps{qg}")
                z_ps = None
                ets = {}
                anchor_mm = None
                dve_set = DVE_KCS[qg]
                for b in range(NB):
                    # pre-block: previous-qg norm ops that must precede
                    # this batch's exps in their engine queues
                    if pend is not None:
                        if b == 1:
                            rz = nrm.tile([128, QG], F32, tag="rz",
                                          name=f"rz{pend['qg']}")
                            nc.vector.reciprocal_approx_fast(
                                rz[:], pend['zbc'][:])
                            pend['rz'] = rz
                        elif b == 2:
                            q0 = pend['qg'] * QG
                            nc.vector.tensor_mul(rt[:, q0:q0 + QG],
                                                 pend['orr'][:],
                                                 pend['rz'][:])
                    # S^T segment (64x128 tiling mode): two chunks
                    # back-to-back so the second pair's LDWs hide in the
                    # first pair's streaming window
                    for sub in range(2):
                        kc = 2 * b + sub
                        if kc >= NKC:
                            continue
                        st = stp.tile([128, 2 * QG], F32, tag="st",
                                      bufs=3, name=f"st_{qg}_{kc}")
                        ktt = kt[kc // 4]
                        ks = slice((kc % 4) * 128, (kc % 4 + 1) * 128)
                        for h in range(2):
                            hp = slice(h * HC, (h + 1) * HC)
                            mm = nc.tensor.matmul(
                                st[:, h * QG:(h + 1) * QG],
                                ktt[hp, ks], qt[qg][hp, :],
                                start=True, stop=True)
                            if kc == 6 and h == 0:
                                anchor_mm = mm
                        et = etp.tile([128, 2 * QG], BF16, tag="et",
                                      bufs=ET_BUFS, name=f"et_{qg}_{kc}")
                        if kc in dve_set:
                            nc.vector._custom_dve(
                                EXP2_OP,
                                out=et[:].bitcast(I16),
                                in0=st[:],
                                in1=ba_t[:], s0=EXP2_A, s1=EXP2_M,
                                imm2=EXP2_C)
                        else:
                            nc.scalar.activation(et[:], st[:], Exp,
                                                 bias=lng_t[:],
                                                 scale=LN2_128)
                        ets[kc] = et
                    # PV segment (128x64 col-tiled mode): two chunk pairs
                    for sub in range(2):
                        pk = 2 * b - 2 * LAG_B + sub
                        if not (0 <= pk < NKC):
                            continue
                        pet = ets[pk]
                        for h in range(2):
                            nc.tensor.matmul(
                                o_ps[h * 64:(h + 1) * 64, :],
                                vall[:, pk, h * 64:(h + 1) * 64],
                                pet[:, h * QG:(h + 1) * QG],
                                start=(pk == 0), stop=(pk == NKC - 1),
                                tile_position=(0, h * 64),
                                skip_group_check=True)
                    # post-block: previous-qg PE/ACT work
                    if pend is not None:
                        if b == 0:
                            pend['zbc'] = emit_zbc(pend['qg'],
                                                   pend['zpair'])
                        elif b == 3:
                            pend['gp'] = emit_outproj_mm(pend['qg'],
                                                         anchor_mm)
                            ob = osbp.tile([128, 512], F32, tag="osb",
                                           name=f"ob_{pend['qg']}")
                            nc.scalar.copy(ob[:], pend['gp'][:])
                            pend['ob'] = ob
                        elif b == 5:
                            emit_out_dma(pend['qg'], pend['ob'])
                            pend = None
                    # Z segment (128x32 col-tiled mode)
                    for zi in zsched.get(b, ()):
                        if z_ps is None:
                            z_ps = zpp.tile([128, QG], F32, tag="z",
                                            name=f"z_ps{qg}")
                        for cg in range(4):
                            pk = 2 * zi + cg // 2
                            hh = cg % 2
                            nc.tensor.matmul(
                                z_ps[cg * 32:(cg + 1) * 32, :],
                                ones_zb[:, 0:32],
                                ets[pk][:, hh * QG:(hh + 1) * QG],
                                start=(zi == 0), stop=(zi == NKC // 2 - 1),
                                tile_position=(0, cg * 32),
                                skip_group_check=True)
                    if qg == 0 and b in (0, 2, 4, 6, 8, 10):
                        emit_proj_chunk(2 + b // 2)
                # tail: evacuate this qg's o_ps / z_ps while the exp
                # engines are idle (no S^T work in the tail batches)
                orr = nrm.tile([128, QG], F32, tag="or", name=f"orr{qg}")
                nc.vector.tensor_copy(orr[:], o_ps[:])
                zsb = nrm.tile([128, QG], BF16, tag="zsb", name=f"zsb{qg}")
                nc.scalar.copy(zsb[:], z_ps[:])
                zpair = emit_zpair(qg, zsb)
                pend = {"qg": qg, "orr": orr, "zpair": zpair}
            # final epilogue
            zbc = emit_zbc(pend['qg'], pend['zpair'])
            rz = nrm.tile([128, QG], F32, tag="rz", name="rz_last")
            nc.vector.reciprocal_approx_fast(rz[:], zbc[:])
            q0 = pend['qg'] * QG
            nc.vector.tensor_mul(rt[:, q0:q0 + QG], pend['orr'][:], rz[:])
            gp = emit_outproj_mm(pend['qg'], None)
            ob = osbp.tile([128, 512], F32, tag="osb", name="ob_last")
            nc.scalar.copy(ob[:], gp[:])
            emit_out_dma(pend['qg'], ob)

    nc.compile()
    return nc


def _prep_in_maps(x, w_qkv, w_out, b_out):
    x = np.asarray(x, dtype=np.float32).reshape(B, NPIX, C)
    w_qkv = np.asarray(w_qkv, dtype=np.float32)
    w_out = np.asarray(w_out, dtype=np.float32)
    b_out = np.asarray(b_out, dtype=np.float32)

    wq = np.concatenate([w_qkv[:, 0:64], w_qkv[:, 192:256]],
                        axis=1) * PRESCALE
    wk = np.concatenate([w_qkv[:, 64:128], w_qkv[:, 256:320]], axis=1)
    wv = np.concatenate([w_qkv[:, 128:192], w_qkv[:, 320:384]], axis=1)
    w4 = np.ascontiguousarray(
        np.concatenate([wq, wk, wv, w_out], axis=1, dtype=np.float32))
    bo = np.ascontiguousarray(b_out.reshape(1, C))

    in_maps = []
    for core in range(N_CORES):
        b, qh = core // 2, core % 2
        xbT = x[b].T                     # [C, NPIX]
        q0 = qh * NQ
        xt = np.ascontiguousarray(
            np.concatenate([xbT[:, q0:], xbT[:, :q0]], axis=1))
        in_maps.append({"xt": xt, "w4": w4, "bo": bo})
    return in_maps


def run(x, w_qkv, w_out, b_out, trace=False, **run_kwargs):
    if "nc" not in _CACHE:
        _CACHE["nc"] = _build_nc()
    nc = _CACHE["nc"]
    in_maps = _prep_in_maps(x, w_qkv, w_out, b_out)
    res = run_bass_kernel_spmd(nc, in_maps, core_ids=list(range(N_CORES)),
                               trace=trace, **run_kwargs)
    out = np.empty((B, NPIX, C), dtype=np.float32)
    for core in range(N_CORES):
        b, qh = core // 2, core % 2
        out[b, qh * NQ:(qh + 1) * NQ, :] = res.results[core]["out"]
    return out.reshape(B, 64, 64, C), res


def kernel(x, w_qkv, w_out, b_out):
    out, _ = run(x, w_qkv, w_out, b_out, trace=False)
    return out


# revision 26
# speedup vs baseline: 1.0113x; 1.0113x over previous
"""Trainium2 Bass kernel for nn_AttentionConv (B=4,H=W=64,C=128,heads=2).

Sharding: 8 cores = (batch b in 0..3) x (query-half qh in 0..1).
Each core computes full attention for its 2048 query pixels of batch b,
over all 4096 keys, both heads, plus the qkv and output 1x1-conv
projections.  No cross-core communication.

v2 structure (vs the 174.9us ones-trick baseline): the PV matmuls for
the two heads are COLUMN-TILED (tile_position (0,0)/(0,64), M=64 each,
own XBUS stream per column group) so both heads' PV runs in ONE
512-cycle pass; the softmax row-sums Z come from a separate 4-way
column-tiled ones-matmul pass (M=32 per tile) every TWO key chunks,
streaming both heads of both chunks concurrently.  PE inner work drops
from 1536 to 1280 cycles/chunk.  has_written clear was verified
REGION-scoped on HW (probe.py), so each column tile manages its own
start= flag independently.

The exp split is unchanged: ScalarE ACT (exact exp, bias=ln G) and the
custom 8-stage DVE op EXP2_PACK_ANT (approx G*2^(T/128) packing bf16
bits through an int16-convert write) split the key chunks; scores
arrive pre-scaled by 128*log2(e)*C^-0.5 folded into wq host-side.

PSUM (8 banks): st pool 3 rotating [128,1024] f32 slots (6 banks);
o_ps [128,512] 1 bank (h0 dims on partitions 0-63, h1 on 64-127);
one shared "z" bank hosting, sequentially per qg: Z accumulation ->
zbc broadcast -> outproj gp -> next qg's Z accumulation.

Normalize for qg n (spread over qg n+1's first slots):
  kc0: orr = DVE copy o_ps (frees the o bank so PV lag stays 4);
       zsb = ACT copy z_ps (frees the z bank)
  kc2: zsum = zsb[0:64] + zsb[64:128] -> bf16 [64,512] (DVE)
  kc4: zbc = col-tiled K=1 ones-matmul pair broadcasting Z_h0 row ->
       partitions 0-63 and Z_h1 row -> 64-127 (PE, bf16)
  kc5: rz = reciprocal_approx_fast(zbc) (DVE)
  kc6: rt[:, qg] = orr * rz (DVE, SBUF x SBUF)
  kc9: outproj gp: K=1 ones x b_out bias init (start=True) + 4
       accumulating [128,128] matmuls (PE)
  kc10: ob = ACT copy(gp); kc12: out DMA.
"""

import numpy as np

import concourse.bass as bass
import concourse.tile as tile
from concourse.tile import add_dep_helper
from concourse import bacc, mybir
from concourse.bass_utils import run_bass_kernel_spmd

import concourse.dve_ops as dops
from concourse.dve_spec import Spec, lower, Src0, Src1, C0, C1, C2, Latch
from concourse.dve_uop import DveOpSpec

F32 = mybir.dt.float32
BF16 = mybir.dt.bfloat16
I16 = mybir.dt.int16

B = 4
C = 128
NPIX = 4096          # 64*64 pixels per batch
NQ = 2048            # queries per core (half batch)
HC = 64              # head dim
KC = 128             # key chunk
NKC = NPIX // KC     # 32
QG = 512             # query group (per head; ST tile packs both heads)
NQG = NQ // QG       # 4
N_CORES = 8
LAG_B = 2            # PV lags the S^T by LAG_B two-chunk batches
ET_BUFS = 16

# --- exp approximation constants (see fit in session notes) ---
EXP2_A = 2.63111957e-3
EXP2_B = 1.00227837
EXP2_C = 16310.5756
EXP2_M = float(1.5 * 2**30)
EXP2_BA = float(np.float32(EXP2_B) / np.float32(EXP2_A))
EXP2_LNG = 0.35459771189588246        # ln(G): ACT-path bias
LN2_128 = float(np.log(2.0) / 128.0)  # ACT-path scale
# host-side score pre-scale folded into wq: T = 128*log2(e)*C^-0.5 * s
PRESCALE = float(128.0 * np.log2(np.e) * C ** -0.5)

OP_NAME = "EXP2_PACK_ANT"

# which key-chunks each query-group runs on DVE (rest go to ACT)
import os as _os
_DVE_ON = _os.environ.get("KERNEL_DVE", "1") == "1"
_QG0_DVE = tuple(range(6, 32, 2))
_QGN_DVE = (1, 3, 5, 7, 9, 11, 15, 17, 19, 21, 23, 25, 27, 29, 31)
DVE_KCS = [
    _QG0_DVE if _DVE_ON else (),
    _QGN_DVE if _DVE_ON else (),
    _QGN_DVE if _DVE_ON else (),
    _QGN_DVE if _DVE_ON else (),
]

_CACHE = {}


def _exp2_ref(in0, in1, s0, s1, imm2):
    f32 = np.float32
    T = in0.astype(f32)
    u = f32(T + f32(s1))
    I = f32(u - f32(s1))
    F = f32(T - I)
    ba = np.asarray(in1, f32).reshape(in0.shape[0], -1)[:, :1]
    r = f32(F + ba)
    p = f32(F * r)
    Q = f32(p * f32(s0))
    return f32(f32(I + f32(imm2)) + Q)


def _register_exp2_op():
    for op in dops.OPS:
        if op.name == OP_NAME:
            return op
    u = Src0 + C1
    I = u - C1
    F = Src0 - I
    r = F + Latch(Src1)
    y = (I + C2) + (F * r) * C0
    spec = Spec(body=y, reference=_exp2_ref)
    row = dops._CUSTOM_DVE_ROW_BASE + len(dops.OPS)
    assert row < 0x20
    shas = {
        ver: DveOpSpec(
            name=OP_NAME, opcode=row, uops=lower(spec, ver=ver), rd1_en=True
        ).sha(ver)
        for ver in ("v3", "v4")
    }
    op = dops.DveOp(OP_NAME, spec, subdim=False, uops_sha=shas)
    dops.OPS.append(op)
    dops._SUB_OPCODE_FOR_NAME[OP_NAME] = row
    dops.CUSTOM_DVE_SPECS[OP_NAME] = spec
    return op


EXP2_OP = _register_exp2_op()


def _build_nc():
    nc = bacc.Bacc("TRN2", target_bir_lowering=False, debug=False)

    xt_d = nc.dram_tensor("xt", [C, NPIX], F32, kind="ExternalInput")
    w4_d = nc.dram_tensor("w4", [C, 512], F32, kind="ExternalInput")
    bo_d = nc.dram_tensor("bo", [1, C], F32, kind="ExternalInput")
    out_d = nc.dram_tensor("out", [NQ, C], F32, kind="ExternalOutput")

    Exp = mybir.ActivationFunctionType.Exp

    with tile.TileContext(nc) as tc:
        with (
            tc.tile_pool(name="const", bufs=1) as const,
            tc.tile_pool(name="stage", bufs=4) as stage,
            tc.tile_pool(name="et", bufs=ET_BUFS) as etp,
            tc.tile_pool(name="nrm", bufs=2) as nrm,
            tc.tile_pool(name="osb", bufs=2) as osbp,
            tc.tile_pool(name="st", bufs=1, space="PSUM") as stp,
            tc.tile_pool(name="op", bufs=1, space="PSUM") as opp,
            tc.tile_pool(name="zp", bufs=1, space="PSUM") as zpp,
        ):
            # ---- persistent SBUF tensors
            xtball = const.tile([C, NPIX], BF16, tag="xtball")
            qt = [const.tile([128, 512], BF16, tag=f"qt{j}", name=f"qt{j}")
                  for j in range(4)]
            kt = [const.tile([128, 512], BF16, tag=f"kt{j}", name=f"kt{j}")
                  for j in range(8)]
            vall = const.tile([128, NKC, 128], BF16, tag="vall")
            rt = const.tile([128, NQ], BF16, tag="rt")
            w4b = const.tile([C, 512], BF16, tag="w4b")
            wqb = w4b[:, 0:128]
            wkb = w4b[:, 128:256]
            wvb = w4b[:, 256:384]
            wob = w4b[:, 384:512]
            ones_zb = const.tile([128, 64], BF16, tag="ones_zb")
            ones1b = const.tile([1, 128], BF16, tag="ones1b")
            bo4b = const.tile([1, 512], BF16, tag="bo4b")
            warm = const.tile([1, 2], F32, tag="warm")
            ba_t = const.tile([128, 1], F32, tag="ba_t")
            lng_t = const.tile([128, 1], F32, tag="lng_t")

            # dummy exp first: loads the ACT table set off the critical path
            nc.vector.memset(warm[:], 0.0)
            nc.scalar.activation(warm[:], warm[:], Exp)
            nc.vector.memset(ba_t[:], EXP2_BA)
            nc.vector.memset(lng_t[:], EXP2_LNG)

            # PE warm-up: dummy matmuls while DMAs run, so the HAM
            # clock-gate reaches K=8/8 before the real matmuls start
            junk = const.tile([C, 512], BF16, tag="junk")
            nc.vector.memset(junk[:], 0.25)
            wst = stp.tile([128, 2 * QG], F32, tag="st", bufs=3, name="warm_st")
            for w in range(6):
                nc.tensor.matmul(wst[:, 0:512], junk[:, 0:128], junk[:],
                                 start=True, stop=True)

            # inputs: f32 HWDGE DMAs (x chunk 0 and weights first), casts
            # spread across DVE / GpSimd so no engine serializes the ramp
            xsall = stage.tile([C, NPIX], F32, tag="xsall", name="xsall")
            xs = [xsall[:, j * 512:(j + 1) * 512] for j in range(8)]
            # two HWDGE queues: SP carries x0 + mid-x, the Activation
            # engine's queue carries the weights + upper-x in parallel
            nc.sync.dma_start(xs[0], xt_d[:, 0:512])
            w32 = stage.tile([C, 512], F32, tag="w32", name="w32")
            nc.scalar.dma_start(w32[:], w4_d[:])
            nc.sync.dma_start(xsall[:, 512:2048], xt_d[:, 512:2048])
            nc.scalar.dma_start(xsall[:, 2048:4096], xt_d[:, 2048:4096])
            nc.vector.tensor_copy(w4b[:], w32[:])
            cast_eng = [nc.vector, nc.vector, nc.gpsimd, nc.gpsimd,
                        nc.gpsimd, nc.gpsimd, nc.gpsimd, nc.gpsimd]
            for j in range(8):
                eng = cast_eng[j]
                dst = xtball[:, j * 512:(j + 1) * 512]
                eng.tensor_copy(dst, xs[j])

            nc.vector.memset(ones_zb[:], 1.0)
            nc.gpsimd.memset(ones1b[:], 1.0)

            def emit_proj_kq(j):
                # QT/KT projections for one 512-pixel chunk; PSUM tiles
                # borrow ST-pool slots.
                p = stp.tile([128, 2 * QG], F32, tag="st", bufs=3,
                             name=f"pkq{j}")
                nc.tensor.matmul(p[:, 0:512], wkb,
                                 xtball[:, j * 512:(j + 1) * 512],
                                 start=True, stop=True)
                if j < 4:  # QT over local queries
                    nc.tensor.matmul(p[:, 512:1024], wqb,
                                     xtball[:, j * 512:(j + 1) * 512],
                                     start=True, stop=True)
                if j % 2 == 1:
                    nc.vector.tensor_copy(kt[j][:], p[:, 0:512])
                else:
                    nc.scalar.copy(kt[j][:], p[:, 0:512])
                if j < 4:
                    nc.vector.tensor_copy(qt[j][:], p[:, 512:1024])

            def emit_proj_v(j):
                pv = stp.tile([128, 2 * QG], F32, tag="st", bufs=3,
                              name=f"pv{j}")
                for kq in range(4):   # V natural per key chunk of 128
                    nc.tensor.matmul(
                        pv[:, kq * 128:(kq + 1) * 128],
                        xtball[:, j * 512 + kq * 128:
                               j * 512 + (kq + 1) * 128],
                        wvb, start=True, stop=True)
                dst = vall[:, 4 * j:4 * j + 4, :]
                src = pv[:, 0:512].rearrange("p (s d) -> p s d", d=128)
                if j % 2 == 1:
                    nc.vector.tensor_copy(dst, src)
                else:
                    nc.scalar.copy(dst, src)

            def emit_proj_chunk(j):
                emit_proj_kq(j)
                emit_proj_v(j)

            # ramp ordering: K/Q of chunk 0 first (feeds first S^T)
            emit_proj_kq(0)
            emit_proj_v(0)
            emit_proj_kq(1)
            emit_proj_v(1)
            bo32 = stage.tile([1, C], F32, tag="bo32")
            nc.sync.dma_start(bo32[:], bo_d[:])
            for r in range(4):
                nc.gpsimd.tensor_copy(bo4b[:, r * 128:(r + 1) * 128],
                                      bo32[:])

            # ---- attention (software-pipelined across query groups) ----
            def emit_zpair(qg, zsb):
                # DMA-realign the four Z rows (lane-locked engines cannot
                # cross partitions): rows {0,64} -> {0,1}, {32,96} -> {32,33}
                zpair = nrm.tile([34, QG], BF16, tag="zpr",
                                 name=f"zpair{qg}")
                nc.sync.dma_start(zpair[0:2, :], zsb[0:65:64, :])
                nc.sync.dma_start(zpair[32:34, :], zsb[32:97:64, :])
                return zpair

            def emit_zbc(qg, zpair):
                # bf16 K=2 ones-matmul: parity sum + broadcast in one pass;
                # Z_h0 -> partitions 0-63, Z_h1 -> 64-127 (concurrent pair
                # on disjoint rows/cols/psum partitions).
                zbc = zpp.tile([128, QG], F32, tag="z", name=f"zbc{qg}")
                nc.tensor.matmul(zbc[0:64, :], ones_zb[0:2, :],
                                 zpair[0:2, :], start=True, stop=True,
                                 tile_position=(0, 0), skip_group_check=True)
                nc.tensor.matmul(zbc[64:128, :], ones_zb[32:34, :],
                                 zpair[32:34, :], start=True, stop=True,
                                 tile_position=(32, 64),
                                 skip_group_check=True)
                return zbc

            def emit_outproj_mm(qg, anchor):
                q0 = qg * QG
                gp = zpp.tile([128, QG], F32, tag="z", name=f"gp{qg}")
                mm = nc.tensor.matmul(gp[:], ones1b[:], bo4b[:],
                                      start=True, stop=False,
                                      skip_group_check=True)
                if anchor is not None:
                    add_dep_helper(mm.ins, anchor.ins, False,
                                   "outproj after current S^T")
                for i in range(4):
                    nc.tensor.matmul(
                        gp[:, i * 128:(i + 1) * 128],
                        rt[:, q0 + i * 128:q0 + (i + 1) * 128],
                        wob, start=False, stop=(i == 3),
                        skip_group_check=True)
                return gp

            def emit_out_dma(qg, ob):
                q0 = qg * QG
                nc.sync.dma_start(
                    out_d[q0:q0 + QG, :].rearrange("(c r) w -> r c w", r=128),
                    ob[:].rearrange("p (c w) -> p c w", w=128))

            # Z-pass schedule: batch index -> list of chunk-pair indices.
            # Pairs 0-5 wait for the z bank to clear the previous qg's
            # zbc/gp tenants (batch 5), then one pair per batch (matching
            # the exp supply rate).
            NB = NKC // 2 + 2   # 16 S^T batches + 2 PV/Z tail batches
            zsched = {7: (0, 1), 8: (2, 3), 9: (4, 5), 10: (6, 7),
                      11: (8, 9), 12: (10,), 13: (11,), 14: (12,),
                      15: (13,), 16: (14,), 17: (15,)}

            # pending state from the previous query group
            pend = None
            for qg in range(NQG):
                o_ps = opp.tile([128, QG], F32, tag="o", name=f"o_ps{qg}")
                z_ps = None
                ets = {}
                anchor_mm = None
                dve_set = DVE_KCS[qg]
                for b in range(NB):
                    # pre-block: previous-qg norm ops that must precede
                    # this batch's exps in their engine queues
                    if pend is not None:
                        if b == 1:
                            rz = nrm.tile([128, QG], F32, tag="rz",
                                          name=f"rz{pend['qg']}")
                            nc.vector.reciprocal_approx_fast(
                                rz[:], pend['zbc'][:])
                            pend['rz'] = rz
                        elif b == 2:
                            q0 = pend['qg'] * QG
                            nc.vector.tensor_mul(rt[:, q0:q0 + QG],
                                                 pend['orr'][:],
                                                 pend['rz'][:])
                    # S^T segment (64x128 tiling mode): two chunks
                    # back-to-back so the second pair's LDWs hide in the
                    # first pair's streaming window
                    for sub in range(2):
                        kc = 2 * b + sub
                        if kc >= NKC:
                            continue
                        st = stp.tile([128, 2 * QG], F32, tag="st",
                                      bufs=3, name=f"st_{qg}_{kc}")
                        ktt = kt[kc // 4]
                        ks = slice((kc % 4) * 128, (kc % 4 + 1) * 128)
                        for h in range(2):
                            hp = slice(h * HC, (h + 1) * HC)
                            mm = nc.tensor.matmul(
                                st[:, h * QG:(h + 1) * QG],
                                ktt[hp, ks], qt[qg][hp, :],
                                start=True, stop=True)
                            if kc == 6 and h == 0:
                                anchor_mm = mm
                        et = etp.tile([128, 2 * QG], BF16, tag="et",
                                      bufs=ET_BUFS, name=f"et_{qg}_{kc}")
                        if kc in dve_set:
                            nc.vector._custom_dve(
                                EXP2_OP,
                                out=et[:].bitcast(I16),
                                in0=st[:],
                                in1=ba_t[:], s0=EXP2_A, s1=EXP2_M,
                                imm2=EXP2_C)
                        else:
                            nc.scalar.activation(et[:], st[:], Exp,
                                                 bias=lng_t[:],
                                                 scale=LN2_128)
                        ets[kc] = et
                    # PV segment (128x64 col-tiled mode): two chunk pairs
                    for sub in range(2):
                        pk = 2 * b - 2 * LAG_B + sub
                        if not (0 <= pk < NKC):
                            continue
                        pet = ets[pk]
                        for h in range(2):
                            nc.tensor.matmul(
                                o_ps[h * 64:(h + 1) * 64, :],
                                vall[:, pk, h * 64:(h + 1) * 64],
                                pet[:, h * QG:(h + 1) * QG],
                                start=(pk == 0), stop=(pk == NKC - 1),
                                tile_position=(0, h * 64),
                                skip_group_check=True)
                    # post-block: previous-qg PE/ACT work
                    if pend is not None:
                        if b == 0:
                            pend['zbc'] = emit_zbc(pend['qg'],
                                                   pend['zpair'])
                        elif b == 5:
                            pend['gp'] = emit_outproj_mm(pend['qg'],
                                                         anchor_mm)
                            ob = osbp.tile([128, 512], F32, tag="osb",
                                           name=f"ob_{pend['qg']}")
                            nc.scalar.copy(ob[:], pend['gp'][:])
                            pend['ob'] = ob
                        elif b == 6:
                            emit_out_dma(pend['qg'], pend['ob'])
                            pend = None
                    # Z segment (128x32 col-tiled mode)
                    for zi in zsched.get(b, ()):
                        if z_ps is None:
                            z_ps = zpp.tile([128, QG], F32, tag="z",
                                            name=f"z_ps{qg}")
                        for cg in range(4):
                            pk = 2 * zi + cg // 2
                            hh = cg % 2
                            nc.tensor.matmul(
                                z_ps[cg * 32:(cg + 1) * 32, :],
                                ones_zb[:, 0:32],
                                ets[pk][:, hh * QG:(hh + 1) * QG],
                                start=(zi == 0), stop=(zi == NKC // 2 - 1),
                                tile_position=(0, cg * 32),
                                skip_group_check=True)
                    if qg == 0 and b in (0, 2, 4, 6, 8, 10):
                        emit_proj_chunk(2 + b // 2)
                # tail: evacuate this qg's o_ps / z_ps while the exp
                # engines are idle (no S^T work in the tail batches)
                orr = nrm.tile([128, QG], F32, tag="or", name=f"orr{qg}")
                nc.vector.tensor_copy(orr[:], o_ps[:])
                zsb = nrm.tile([128, QG], BF16, tag="zsb", name=f"zsb{qg}")
                nc.scalar.copy(zsb[:], z_ps[:])
                zpair = emit_zpair(qg, zsb)
                pend = {"qg": qg, "orr": orr, "zpair": zpair}
            # final epilogue
            zbc = emit_zbc(pend['qg'], pend['zpair'])
            rz = nrm.tile([128, QG], F32, tag="rz", name="rz_last")
            nc.vector.reciprocal_approx_fast(rz[:], zbc[:])
            q0 = pend['qg'] * QG
            nc.vector.tensor_mul(rt[:, q0:q0 + QG], pend['orr'][:], rz[:])
            gp = emit_outproj_mm(pend['qg'], None)
            ob = osbp.tile([128, 512], F32, tag="osb", name="ob_last")
            nc.scalar.copy(ob[:], gp[:])
            emit_out_dma(pend['qg'], ob)

    nc.compile()
    return nc


def _prep_in_maps(x, w_qkv, w_out, b_out):
    x = np.asarray(x, dtype=np.float32).reshape(B, NPIX, C)
    w_qkv = np.asarray(w_qkv, dtype=np.float32)
    w_out = np.asarray(w_out, dtype=np.float32)
    b_out = np.asarray(b_out, dtype=np.float32)

    wq = np.concatenate([w_qkv[:, 0:64], w_qkv[:, 192:256]],
                        axis=1) * PRESCALE
    wk = np.concatenate([w_qkv[:, 64:128], w_qkv[:, 256:320]], axis=1)
    wv = np.concatenate([w_qkv[:, 128:192], w_qkv[:, 320:384]], axis=1)
    w4 = np.ascontiguousarray(
        np.concatenate([wq, wk, wv, w_out], axis=1, dtype=np.float32))
    bo = np.ascontiguousarray(b_out.reshape(1, C))

    in_maps = []
    for core in range(N_CORES):
        b, qh = core // 2, core % 2
        xbT = x[b].T                     # [C, NPIX]
        q0 = qh * NQ
        xt = np.ascontiguousarray(
            np.concatenate([xbT[:, q0:], xbT[:, :q0]], axis=1))
        in_maps.append({"xt": xt, "w4": w4, "bo": bo})
    return in_maps


def run(x, w_qkv, w_out, b_out, trace=False, **run_kwargs):
    if "nc" not in _CACHE:
        _CACHE["nc"] = _build_nc()
    nc = _CACHE["nc"]
    in_maps = _prep_in_maps(x, w_qkv, w_out, b_out)
    res = run_bass_kernel_spmd(nc, in_maps, core_ids=list(range(N_CORES)),
                               trace=trace, **run_kwargs)
    out = np.empty((B, NPIX, C), dtype=np.float32)
    for core in range(N_CORES):
        b, qh = core // 2, core % 2
        out[b, qh * NQ:(qh + 1) * NQ, :] = res.results[core]["out"]
    return out.reshape(B, 64, 64, C), res


def kernel(x, w_qkv, w_out, b_out):
    out, _ = run(x, w_qkv, w_out, b_out, trace=False)
    return out
